# revision 12
# baseline (speedup 1.0000x reference)
"""NonLocalBlock (dense self-attention over 64x64 pixels) on 8 Trainium2 cores.

Sharding: 8 cores = 4 batches x 2 query-halves of 2048 pixels each.
Each core holds the full x[b] (for keys/values) plus its query slice, and
computes its [C, 2048] slab of the output, residual included. The host
gathers the 8 slabs.

v2 changes over the first working kernel (235us):
  - x, QKV weights, ones ship as bf16 (halves the 8MB head DMA that kept
    PE idle for 18.7us), and ~48 tiny warmup matmuls on the ones tile run
    during the load so the HAM clock gate is at 2.4GHz when QKV starts
    (it measured 1.2GHz until 34us before).
  - theta/phi biases are never applied on device: expanding
    (th+bt)(ph+bp) gives a per-key term c[k] = (W_phi^T b_th)^T x_k
    (computed as a 257th output column of the g matmul, extracted with an
    ACT copy that also folds the -50 softmax shift), a per-QUERY term
    that softmax cancels exactly (dropped), and a constant (dropped).
  - exp output eT is bf16; the softmax denominator is a 31-add binary
    tree on DVE (bf16, 2x rate) + ONE ones-matmul per q-tile. The
    gpsimd pair/quad sums of v1 (96 adds x 1.3us = 126us busy, the
    engine was 56% occupied and stalled PE mid-tile) are gone; gpsimd
    now does nothing.
  - 1/s via reciprocal_approx_fast (51-ULP, ~5x faster than the 3.4us
    iterative reciprocal), epilogue is mul + residual-add only (bias
    pre-folded into the resident x while attention runs).
  - the per-qtile epilogue (sums matmul, out-proj, normalize, residual,
    store) is emitted interleaved into the first steps of the NEXT
    q-tile's score loop so PE never drains at tile boundaries.

Per-core math (Q=theta over its 2048 queries; phi/g over all 4096 keys):
  fT[k,q] = sum_o phi[o,k] Q[o,q]        f32r matmuls
  eT      = exp(fT + c[k] - 50)          bf16 out
  y~T[o,q]= sum_k g[k,o] eT[k,q]         bf16 matmuls, fp32 PSUM
  s[q]    = ones-matmul over tree-summed eT
  out[c,q]= (W_out^T y~T)[c,q] / s[q] + xb[c,q],  xb = x + b_out + W_out b_g
"""

import json

import numpy as np

B, C, HH, WW = 4, 512, 64, 64
CI = 256
N = HH * WW          # 4096 pixels
NQ = N // 2          # queries per core
P = 128
QT = 512             # q-tile width
NQT = NQ // QT       # 4 q-tiles per core
NKC = N // P         # 32 key chunks
NCC = C // P         # 4 channel chunks
NOC = CI // P        # 2 inter-channel chunks
SHIFT = 50.0
LAG = 3              # PV trails scores by this many chunks
NWARM = 20           # HAM warmup matmuls during the input DMA

_cache: dict = {}


def _install_bir_patch():
    """This walrus build rejects >1 sync-wait per instruction; Tile's tail
    drain (and some first-consumer instructions) carry several. Split the
    extras onto preceding single-wait EventSemaphore instructions."""
    import concourse.bass_utils as bass_utils
    import concourse.bass2jax as bass2jax

    if getattr(bass_utils.compile_bir_kernel, "_wait_split_patch", False):
        return
    orig = bass_utils.compile_bir_kernel

    def _split(bir_json: bytes) -> bytes:
        d = json.loads(bir_json)
        changed = False
        for fn in d.get("functions", []):
            for bb in fn.get("blocks", []):
                new = []
                for ins in bb.get("instructions", []):
                    si = ins.get("sync_info")
                    waits = (si or {}).get("on_wait") or []
                    if len(waits) > 1:
                        changed = True
                        for k, w in enumerate(waits[:-1]):
                            new.append({
                                "debug": ins.get("debug", 0),
                                "engine": ins["engine"],
                                "ins": [],
                                "outs": [],
                                "name": f"{ins['name']}-w{k}",
                                "opcode": "EventSemaphore",
                                "sync_info": {"on_update": [], "on_wait": [w]},
                            })
                        si["on_wait"] = [waits[-1]]
                    new.append(ins)
                bb["instructions"] = new
        return json.dumps(d).encode() if changed else bir_json

    def patched(bir_json, tmpdir, neff_name="file.neff"):
        return orig(_split(bir_json), tmpdir, neff_name)

    patched._wait_split_patch = True
    bass_utils.compile_bir_kernel = patched
    bass2jax.compile_bir_kernel = patched


def _build_nc():
    import concourse.bass as bass
    import concourse.mybir as mybir
    from concourse import tile

    dt = mybir.dt
    f32, f32r, bf16, f16 = dt.float32, dt.float32r, dt.bfloat16, dt.float16
    Exp = mybir.ActivationFunctionType.Exp
    Ln = mybir.ActivationFunctionType.Ln
    Copy = mybir.ActivationFunctionType.Copy

    nc = bass.Bass("TRN2", target_bir_lowering=False, debug=False)

    xf_d = nc.dram_tensor("xf", [C, N], f16, kind="ExternalInput")
    wqp_d = nc.dram_tensor("wqp", [C, 2 * CI], f16, kind="ExternalInput")
    wg_d = nc.dram_tensor("wg", [C, CI + 1], f16, kind="ExternalInput")
    wo_d = nc.dram_tensor("wo", [CI, C], bf16, kind="ExternalInput")
    bo_d = nc.dram_tensor("bo", [P, NCC], f32, kind="ExternalInput")
    ones_d = nc.dram_tensor("ones", [P, P], bf16, kind="ExternalInput")
    out_d = nc.dram_tensor("out", [C, NQ], f32, kind="ExternalOutput")

    with tile.TileContext(nc) as tc:
        with (
            tc.tile_pool(name="wts", bufs=1) as wpool,
            tc.tile_pool(name="persist", bufs=1) as ppool,
        ):
            wqp_s = wpool.tile([P, NCC, 2 * CI], f16)
            wg_s = wpool.tile([P, NCC, CI + 1], f16)
            wo_s = wpool.tile([P, NOC, C], bf16)
            bo_s = wpool.tile([P, NCC], f32)
            ones_s = wpool.tile([P, P], bf16)
            wrm_s = wpool.tile([P, P], bf16)
            lnb_s = wpool.tile([P, 1], f32)

            q_s = ppool.tile([P, NOC, NQ], f16)
            phi_s = ppool.tile([P, NOC, N], f16)
            g_s = ppool.tile([P, NKC, CI], bf16)
            c_s = ppool.tile([P, NKC], f32)

            all_ps = tc.tile_pool(name="all_ps", bufs=1, space="PSUM")
            ctx_ps = all_ps.__enter__()

            # ---- input DMAs, warmup, QKV phase ----
            xin_cm = tc.tile_pool(name="xin", bufs=1)
            xpool = xin_cm.__enter__()
            if True:
                xf_c = [xpool.tile([P, N], f16, tag=f"xf{kc}", name=f"xf_c{kc}") for kc in range(NCC)]
                xq_c = [t[:, :NQ] for t in xf_c]
                xf_r = xf_d.ap().rearrange("(kc p) n -> kc p n", p=P)
                # ones first (warmup fodder), then the query halves
                nc.sync.dma_start(ones_s[:], ones_d.ap())
                nc.sync.dma_start(wqp_s[:], wqp_d.ap().rearrange("(kc p) m -> p kc m", p=P))
                for kc in range(NCC):
                    nc.sync.dma_start(xf_c[kc][:, :NQ], xf_r[kc][:, :NQ])
                for kc in range(NCC):
                    nc.sync.dma_start(xf_c[kc][:, NQ:], xf_r[kc][:, NQ:])
                nc.sync.dma_start(wg_s[:], wg_d.ap().rearrange("(kc p) o -> p kc o", p=P))
                nc.sync.dma_start(wo_s[:], wo_d.ap().rearrange("(oc p) c -> p oc c", p=P))
                nc.sync.dma_start(bo_s[:], bo_d.ap())

                # warm the HAM clock gate while x streams in (memset
                # fodder: no DMA dependency, PE busy from ~0)
                nc.vector.memset(wrm_s[:], 0.0)
                nc.vector.memset(lnb_s[:], -44.0 * 0.6931471805599453)
                for i in range(NWARM):
                    wp = ctx_ps.tile([P, QT], f32, tag="fps", bufs=2, name="warm")
                    nc.tensor.matmul(wp[:, :P], wrm_s[:], wrm_s[:],
                                     start=True, stop=True)

                # Q (needs only the query halves), raw: bias folded away
                for mc in range(NOC):
                    for t in range(NQ // QT):
                        ps = ctx_ps.tile([P, QT], f32, tag="qkv", bufs=3, name="ps")
                        for kc in range(NCC):
                            nc.tensor.matmul(
                                ps[:],
                                wqp_s[:, kc, mc * P:(mc + 1) * P],
                                xq_c[kc][:, t * QT:(t + 1) * QT],
                                start=(kc == 0),
                                stop=(kc == NCC - 1),
                            )
                        nc.scalar.activation(
                            q_s[:, mc, t * QT:(t + 1) * QT], ps[:], Copy)
                for mc in range(NOC):
                    for t in range(N // QT):
                        ps = ctx_ps.tile([P, QT], f32, tag="qkv", bufs=3, name="ps")
                        for kc in range(NCC):
                            nc.tensor.matmul(
                                ps[:],
                                wqp_s[:, kc, (NOC + mc) * P:(NOC + mc + 1) * P],
                                xf_c[kc][:, t * QT:(t + 1) * QT],
                                start=(kc == 0),
                                stop=(kc == NCC - 1),
                            )
                        nc.scalar.activation(
                            phi_s[:, mc, t * QT:(t + 1) * QT], ps[:], Copy)

                # g in natural [pixel, o] layout; col CI is c[k] (key-side
                # score bias), extracted with the -SHIFT fold
                for kc in range(NKC):
                    ps = ctx_ps.tile([P, QT], f32, tag="qkv", bufs=3, name="ps")[:, :CI + 1]
                    for cc in range(NCC):
                        nc.tensor.matmul(
                            ps[:],
                            xf_c[cc][:, kc * P:(kc + 1) * P],
                            wg_s[:, cc, :],
                            start=(cc == 0),
                            stop=(cc == NCC - 1),
                        )
                    nc.vector.tensor_copy(g_s[:, kc, :], ps[:, :CI])
                    nc.scalar.activation(
                        c_s[:, kc:kc + 1], ps[:, CI:CI + 1], Copy, bias=-SHIFT)

                # pre-bias the resident x query half for the residual path;
                # runs on DVE once all QKV matmul reads of xf are done
                for cc in range(NCC):
                    nc.vector.tensor_scalar_add(
                        xq_c[cc][:], xq_c[cc][:], bo_s[:, cc:cc + 1])

            # ---- attention: one global software-pipelined stream ----
            with (
                tc.tile_pool(name="attn_sb", bufs=3) as apool,
                tc.tile_pool(name="epi_sb", bufs=2) as epool,
            ):
                TOT = NQT * NKC
                exps = {}          # global chunk idx -> eT tile
                tree = [dict() for _ in range(5)]  # per qtile rebuilt
                yaccs = {}
                E_tiles = {}
                saccs = {}
                recips = {}
                yTs = {}
                wys = {}

                def scores_exp(Gi):
                    qt, kc = divmod(Gi, NKC)
                    qsl = slice(qt * QT, (qt + 1) * QT)
                    fp = ctx_ps.tile([P, QT], f32, tag="fps", bufs=2, name="fp")
                    for oc in range(NOC):
                        nc.tensor.matmul(
                            fp[:],
                            phi_s[:, oc, kc * P:(kc + 1) * P],
                            q_s[:, oc, qsl],
                            start=(oc == 0),
                            stop=(oc == NOC - 1),
                        )
                    eT = apool.tile([P, QT], bf16, tag="eT", bufs=6)
                    nc.scalar.activation(eT[:], fp[:], Exp, bias=c_s[:, kc:kc + 1])
                    exps[Gi] = eT
                    # denominator tree on DVE (bf16)
                    node, lvl = eT, 0
                    idx = kc
                    while idx % 2 == 1:
                        if lvl == 4 and qt == NQT - 1:
                            # last q-tile: keep the 16-chunk halves separate
                            # so the first denominator matmul can run ~3.5us
                            # before the tile ends (shorter serial tail)
                            E_tiles[qt] = (tree[4].pop(0), node)
                            return
                        sib = tree[lvl].pop(idx - 1) if lvl else exps[Gi - 1]
                        nxt = apool.tile([P, QT], bf16, tag=f"t{lvl}", bufs=3)
                        nc.vector.tensor_add(out=nxt[:], in0=sib[:], in1=node[:])
                        node, lvl, idx = nxt, lvl + 1, idx // 2
                        if lvl == 5:
                            E_tiles[qt] = node
                            return
                    tree[lvl][idx] = node

                def pv(Gi):
                    qt, kc = divmod(Gi, NKC)
                    if kc == 0:
                        yaccs[qt] = ctx_ps.tile(
                            [P, NOC, QT], f32, tag="yacc", bufs=1, name="y_acc")
                    y_acc = yaccs[qt]
                    eT = exps.pop(Gi)
                    for oc in range(NOC):
                        nc.tensor.matmul(
                            y_acc[:, oc],
                            g_s[:, kc, oc * P:(oc + 1) * P],
                            eT[:],
                            start=(kc == 0),
                            stop=(kc == NKC - 1),
                            skip_group_check=True,
                        )
                    if kc == NKC - 1:
                        # release the PSUM pair promptly: cast both halves
                        yT = epool.tile([P, NOC, QT], bf16, tag="yT")
                        for oc in range(NOC):
                            nc.vector.tensor_copy(yT[:, oc], y_acc[:, oc])
                        yTs[qt] = yT

                def sums(qt):
                    s_acc = ctx_ps.tile([P, QT], f32, tag="sacc", bufs=1, name="s_acc")
                    E = E_tiles.pop(qt)
                    if isinstance(E, tuple):
                        for j, Ej in enumerate(E):
                            nc.tensor.matmul(
                                s_acc[:], ones_s[:], Ej[:],
                                start=(j == 0), stop=(j == len(E) - 1),
                                skip_group_check=True)
                    else:
                        nc.tensor.matmul(
                            s_acc[:], ones_s[:], E[:],
                            start=True, stop=True, skip_group_check=True)
                    saccs[qt] = s_acc
                    # 1/s = exp(-ln(s*2^-44) - 44*ln2); the 2^-44 scale is
                    # exact and recenters ln's input range (s reaches ~1e30,
                    # where the ACT ln spline misbehaves)
                    ln_t = epool.tile([P, QT], f32, tag="ln_t")
                    nc.scalar.activation(ln_t[:], s_acc[:], Ln, scale=2.0 ** -44)
                    recip = epool.tile([P, QT], f32, tag="recip")
                    nc.scalar.activation(recip[:], ln_t[:], Exp, scale=-1.0,
                                         bias=lnb_s[:, 0:1])
                    recips[qt] = recip

                def wy_out(qt, cc):
                    qsl = slice(qt * QT, (qt + 1) * QT)
                    wy = ctx_ps.tile([P, QT], f32, tag="qkv", bufs=3, name="wy")
                    yT = yTs[qt]
                    for oc in range(NOC):
                        nc.tensor.matmul(
                            wy[:],
                            wo_s[:, oc, cc * P:(cc + 1) * P],
                            yT[:, oc],
                            start=(oc == 0),
                            stop=(oc == NOC - 1),
                            skip_group_check=True,
                        )
                    ot = epool.tile([P, QT], f32, tag="ot", bufs=3)
                    nc.vector.tensor_mul(out=ot[:], in0=wy[:], in1=recips[qt][:])
                    eng = nc.gpsimd if qt == NQT - 1 else nc.vector
                    eng.tensor_add(
                        out=ot[:], in0=ot[:], in1=xq_c[cc][:, qsl])
                    nc.sync.dma_start(out_d.ap()[cc * P:(cc + 1) * P, qsl], ot[:])
                    if cc == NCC - 1:
                        yTs.pop(qt)
                        recips.pop(qt)
                        saccs.pop(qt, None)

                for G in range(TOT + LAG + 8):
                    if LAG <= G < TOT + LAG:
                        pv(G - LAG)
                    # epilogue of qtile qt, interleaved after its scores end
                    # at S = qt*NKC + (NKC-1): sums at S+3, wy at S+4..S+7
                    off = (G - (NKC - 1)) % NKC
                    qt_e = (G - (NKC - 1)) // NKC
                    if 0 <= qt_e < NQT:
                        if off == 3:
                            sums(qt_e)
                        elif 4 <= off <= 7:
                            wy_out(qt_e, off - 4)
                    if G < TOT:
                        scores_exp(G)
            all_ps.__exit__(None, None, None)
            xin_cm.__exit__(None, None, None)
    return nc


def _get_nc():
    if "nc" not in _cache:
        _install_bir_patch()
        _cache["nc"] = _build_nc()
    return _cache["nc"]


def kernel(x, w_theta, b_theta, w_phi, b_phi, w_g, b_g, w_out, b_out,
           _trace=False):
    import ml_dtypes
    from concourse.bass_utils import run_bass_kernel_spmd

    bf = ml_dtypes.bfloat16
    x = np.asarray(x, dtype=np.float32)
    w_theta = np.asarray(w_theta, dtype=np.float32)
    b_theta = np.asarray(b_theta, dtype=np.float32)
    w_phi = np.asarray(w_phi, dtype=np.float32)
    b_phi = np.asarray(b_phi, dtype=np.float32)
    w_g = np.asarray(w_g, dtype=np.float32)
    b_g = np.asarray(b_g, dtype=np.float32)
    w_out = np.asarray(w_out, dtype=np.float32)
    b_out = np.asarray(b_out, dtype=np.float32)

    nc = _get_nc()

    xf = np.ascontiguousarray(x.reshape(B, C, N).astype(np.float16))
    wqp = np.ascontiguousarray(
        np.concatenate([w_theta, w_phi], axis=0).T.astype(np.float16))  # [C, 2CI]
    w_c = w_phi.T @ b_theta                                     # [C]
    wg = np.ascontiguousarray(
        np.concatenate([w_g.T, w_c[:, None]], axis=1).astype(np.float16))  # [C, CI+1]
    wo = np.ascontiguousarray(w_out.T.astype(bf))               # [CI, C] bf16
    bo_eff = b_out + w_out @ b_g
    bo = np.ascontiguousarray(bo_eff.reshape(NCC, P).T)         # [P, NCC]
    ones = np.ones((P, P), dtype=bf)

    shared = {"wqp": wqp, "wg": wg, "wo": wo, "bo": bo, "ones": ones}
    in_maps = []
    for core in range(8):
        b, h = divmod(core, 2)
        # query half first; attention is permutation-invariant over keys
        xperm = np.concatenate(
            [xf[b][:, h * NQ:(h + 1) * NQ], xf[b][:, (1 - h) * NQ:(2 - h) * NQ]],
            axis=1)
        in_maps.append({"xf": np.ascontiguousarray(xperm), **shared})

    res = run_bass_kernel_spmd(nc, in_maps, core_ids=list(range(8)), trace=_trace)
    _cache["last_results"] = res

    out = np.empty((B, C, N), dtype=np.float32)
    for core in range(8):
        b, h = divmod(core, 2)
        out[b][:, h * NQ:(h + 1) * NQ] = res.results[core]["out"]
    return out.reshape(B, C, HH, WW)


# revision 13
# speedup vs baseline: 1.0054x; 1.0054x over previous
"""NonLocalBlock (dense self-attention over 64x64 pixels) on 8 Trainium2 cores.

Sharding: 8 cores = 4 batches x 2 query-halves of 2048 pixels each.
Each core holds the full x[b] (for keys/values) plus its query slice, and
computes its [C, 2048] slab of the output, residual included. The host
gathers the 8 slabs.

v2 changes over the first working kernel (235us):
  - x, QKV weights, ones ship as bf16 (halves the 8MB head DMA that kept
    PE idle for 18.7us), and ~48 tiny warmup matmuls on the ones tile run
    during the load so the HAM clock gate is at 2.4GHz when QKV starts
    (it measured 1.2GHz until 34us before).
  - theta/phi biases are never applied on device: expanding
    (th+bt)(ph+bp) gives a per-key term c[k] = (W_phi^T b_th)^T x_k
    (computed as a 257th output column of the g matmul, extracted with an
    ACT copy that also folds the -50 softmax shift), a per-QUERY term
    that softmax cancels exactly (dropped), and a constant (dropped).
  - exp output eT is bf16; the softmax denominator is a 31-add binary
    tree on DVE (bf16, 2x rate) + ONE ones-matmul per q-tile. The
    gpsimd pair/quad sums of v1 (96 adds x 1.3us = 126us busy, the
    engine was 56% occupied and stalled PE mid-tile) are gone; gpsimd
    now does nothing.
  - 1/s via reciprocal_approx_fast (51-ULP, ~5x faster than the 3.4us
    iterative reciprocal), epilogue is mul + residual-add only (bias
    pre-folded into the resident x while attention runs).
  - the per-qtile epilogue (sums matmul, out-proj, normalize, residual,
    store) is emitted interleaved into the first steps of the NEXT
    q-tile's score loop so PE never drains at tile boundaries.

Per-core math (Q=theta over its 2048 queries; phi/g over all 4096 keys):
  fT[k,q] = sum_o phi[o,k] Q[o,q]        f32r matmuls
  eT      = exp(fT + c[k] - 50)          bf16 out
  y~T[o,q]= sum_k g[k,o] eT[k,q]         bf16 matmuls, fp32 PSUM
  s[q]    = ones-matmul over tree-summed eT
  out[c,q]= (W_out^T y~T)[c,q] / s[q] + xb[c,q],  xb = x + b_out + W_out b_g
"""

import json

import numpy as np

B, C, HH, WW = 4, 512, 64, 64
CI = 256
N = HH * WW          # 4096 pixels
NQ = N // 2          # queries per core
P = 128
QT = 512             # q-tile width
NQT = NQ // QT       # 4 q-tiles per core
NKC = N // P         # 32 key chunks
NCC = C // P         # 4 channel chunks
NOC = CI // P        # 2 inter-channel chunks
SHIFT = 50.0
LAG = 3              # PV trails scores by this many chunks
NWARM = 40           # HAM warmup matmuls during the input DMA

_cache: dict = {}


def _install_bir_patch():
    """This walrus build rejects >1 sync-wait per instruction; Tile's tail
    drain (and some first-consumer instructions) carry several. Split the
    extras onto preceding single-wait EventSemaphore instructions."""
    import concourse.bass_utils as bass_utils
    import concourse.bass2jax as bass2jax

    if getattr(bass_utils.compile_bir_kernel, "_wait_split_patch", False):
        return
    orig = bass_utils.compile_bir_kernel

    def _split(bir_json: bytes) -> bytes:
        d = json.loads(bir_json)
        changed = False
        for fn in d.get("functions", []):
            for bb in fn.get("blocks", []):
                new = []
                for ins in bb.get("instructions", []):
                    si = ins.get("sync_info")
                    waits = (si or {}).get("on_wait") or []
                    if len(waits) > 1:
                        changed = True
                        for k, w in enumerate(waits[:-1]):
                            new.append({
                                "debug": ins.get("debug", 0),
                                "engine": ins["engine"],
                                "ins": [],
                                "outs": [],
                                "name": f"{ins['name']}-w{k}",
                                "opcode": "EventSemaphore",
                                "sync_info": {"on_update": [], "on_wait": [w]},
                            })
                        si["on_wait"] = [waits[-1]]
                    new.append(ins)
                bb["instructions"] = new
        return json.dumps(d).encode() if changed else bir_json

    def patched(bir_json, tmpdir, neff_name="file.neff"):
        return orig(_split(bir_json), tmpdir, neff_name)

    patched._wait_split_patch = True
    bass_utils.compile_bir_kernel = patched
    bass2jax.compile_bir_kernel = patched


def _build_nc():
    import concourse.bass as bass
    import concourse.mybir as mybir
    from concourse import tile

    dt = mybir.dt
    f32, f32r, bf16, f16 = dt.float32, dt.float32r, dt.bfloat16, dt.float16
    Exp = mybir.ActivationFunctionType.Exp
    Ln = mybir.ActivationFunctionType.Ln
    Copy = mybir.ActivationFunctionType.Copy

    nc = bass.Bass("TRN2", target_bir_lowering=False, debug=False)

    xf_d = nc.dram_tensor("xf", [C, N], f16, kind="ExternalInput")
    wqp_d = nc.dram_tensor("wqp", [C, 2 * CI], f16, kind="ExternalInput")
    wg_d = nc.dram_tensor("wg", [C, CI + 1], f16, kind="ExternalInput")
    wo_d = nc.dram_tensor("wo", [CI, C], bf16, kind="ExternalInput")
    bo_d = nc.dram_tensor("bo", [P, NCC], f32, kind="ExternalInput")
    ones_d = nc.dram_tensor("ones", [P, P], bf16, kind="ExternalInput")
    out_d = nc.dram_tensor("out", [C, NQ], f32, kind="ExternalOutput")

    with tile.TileContext(nc) as tc:
        with (
            tc.tile_pool(name="wts", bufs=1) as wpool,
            tc.tile_pool(name="persist", bufs=1) as ppool,
        ):
            wqp_s = wpool.tile([P, NCC, 2 * CI], f16)
            wg_s = wpool.tile([P, NCC, CI + 1], f16)
            wo_s = wpool.tile([P, NOC, C], bf16)
            bo_s = wpool.tile([P, NCC], f32)
            ones_s = wpool.tile([P, P], bf16)
            wrm_s = wpool.tile([P, P], bf16)
            lnb_s = wpool.tile([P, 1], f32)

            q_s = ppool.tile([P, NOC, NQ], f16)
            phi_s = ppool.tile([P, NOC, N], f16)
            g_s = ppool.tile([P, NKC, CI], bf16)
            c_s = ppool.tile([P, NKC], f32)

            all_ps = tc.tile_pool(name="all_ps", bufs=1, space="PSUM")
            ctx_ps = all_ps.__enter__()

            # ---- input DMAs, warmup, QKV phase ----
            xin_cm = tc.tile_pool(name="xin", bufs=1)
            xpool = xin_cm.__enter__()
            if True:
                xf_c = [xpool.tile([P, N], f16, tag=f"xf{kc}", name=f"xf_c{kc}") for kc in range(NCC)]
                xq_c = [t[:, :NQ] for t in xf_c]
                xf_r = xf_d.ap().rearrange("(kc p) n -> kc p n", p=P)
                # ones first (warmup fodder), then the query halves
                nc.sync.dma_start(ones_s[:], ones_d.ap())
                nc.sync.dma_start(wqp_s[:], wqp_d.ap().rearrange("(kc p) m -> p kc m", p=P))
                for kc in range(NCC):
                    nc.sync.dma_start(xf_c[kc][:, :NQ], xf_r[kc][:, :NQ])
                for kc in range(NCC):
                    nc.sync.dma_start(xf_c[kc][:, NQ:], xf_r[kc][:, NQ:])
                nc.sync.dma_start(wg_s[:], wg_d.ap().rearrange("(kc p) o -> p kc o", p=P))
                nc.sync.dma_start(wo_s[:], wo_d.ap().rearrange("(oc p) c -> p oc c", p=P))
                nc.sync.dma_start(bo_s[:], bo_d.ap())

                # warm the HAM clock gate while x streams in (memset
                # fodder: no DMA dependency, PE busy from ~0)
                nc.vector.memset(wrm_s[:], 0.0)
                nc.vector.memset(lnb_s[:], -44.0 * 0.6931471805599453)
                for i in range(NWARM):
                    wp = ctx_ps.tile([P, QT], f32, tag="fps", bufs=2, name="warm")
                    nc.tensor.matmul(wp[:, :P], wrm_s[:], wrm_s[:],
                                     start=True, stop=True)

                # Q (needs only the query halves), raw: bias folded away
                for mc in range(NOC):
                    for t in range(NQ // QT):
                        ps = ctx_ps.tile([P, QT], f32, tag="qkv", bufs=3, name="ps")
                        for kc in range(NCC):
                            nc.tensor.matmul(
                                ps[:],
                                wqp_s[:, kc, mc * P:(mc + 1) * P],
                                xq_c[kc][:, t * QT:(t + 1) * QT],
                                start=(kc == 0),
                                stop=(kc == NCC - 1),
                            )
                        nc.scalar.activation(
                            q_s[:, mc, t * QT:(t + 1) * QT], ps[:], Copy)
                for mc in range(NOC):
                    for t in range(N // QT):
                        ps = ctx_ps.tile([P, QT], f32, tag="qkv", bufs=3, name="ps")
                        for kc in range(NCC):
                            nc.tensor.matmul(
                                ps[:],
                                wqp_s[:, kc, (NOC + mc) * P:(NOC + mc + 1) * P],
                                xf_c[kc][:, t * QT:(t + 1) * QT],
                                start=(kc == 0),
                                stop=(kc == NCC - 1),
                            )
                        nc.scalar.activation(
                            phi_s[:, mc, t * QT:(t + 1) * QT], ps[:], Copy)

                # g in natural [pixel, o] layout; col CI is c[k] (key-side
                # score bias), extracted with the -SHIFT fold
                for kc in range(NKC):
                    ps = ctx_ps.tile([P, QT], f32, tag="qkv", bufs=3, name="ps")[:, :CI + 1]
                    for cc in range(NCC):
                        nc.tensor.matmul(
                            ps[:],
                            xf_c[cc][:, kc * P:(kc + 1) * P],
                            wg_s[:, cc, :],
                            start=(cc == 0),
                            stop=(cc == NCC - 1),
                        )
                    nc.vector.tensor_copy(g_s[:, kc, :], ps[:, :CI])
                    nc.scalar.activation(
                        c_s[:, kc:kc + 1], ps[:, CI:CI + 1], Copy, bias=-SHIFT)

                # pre-bias the resident x query half for the residual path;
                # runs on DVE once all QKV matmul reads of xf are done
                for cc in range(NCC):
                    nc.vector.tensor_scalar_add(
                        xq_c[cc][:], xq_c[cc][:], bo_s[:, cc:cc + 1])

            # ---- attention: one global software-pipelined stream ----
            with (
                tc.tile_pool(name="attn_sb", bufs=3) as apool,
                tc.tile_pool(name="epi_sb", bufs=2) as epool,
            ):
                TOT = NQT * NKC
                exps = {}          # global chunk idx -> eT tile
                tree = [dict() for _ in range(5)]  # per qtile rebuilt
                yaccs = {}
                E_tiles = {}
                saccs = {}
                recips = {}
                yTs = {}
                wys = {}

                def scores_exp(Gi):
                    qt, kc = divmod(Gi, NKC)
                    qsl = slice(qt * QT, (qt + 1) * QT)
                    fp = ctx_ps.tile([P, QT], f32, tag="fps", bufs=2, name="fp")
                    for oc in range(NOC):
                        nc.tensor.matmul(
                            fp[:],
                            phi_s[:, oc, kc * P:(kc + 1) * P],
                            q_s[:, oc, qsl],
                            start=(oc == 0),
                            stop=(oc == NOC - 1),
                        )
                    eT = apool.tile([P, QT], bf16, tag="eT", bufs=6)
                    nc.scalar.activation(eT[:], fp[:], Exp, bias=c_s[:, kc:kc + 1])
                    exps[Gi] = eT
                    # denominator tree on DVE (bf16)
                    node, lvl = eT, 0
                    idx = kc
                    while idx % 2 == 1:
                        if lvl == 4 and qt == NQT - 1:
                            # last q-tile: the low 16-chunk half was already
                            # fed to the denominator accumulator by sums_low;
                            # keep only the high half for the closing matmul
                            E_tiles[qt] = node
                            return
                        sib = tree[lvl].pop(idx - 1) if lvl else exps[Gi - 1]
                        nxt = apool.tile([P, QT], bf16, tag=f"t{lvl}", bufs=3)
                        nc.vector.tensor_add(out=nxt[:], in0=sib[:], in1=node[:])
                        node, lvl, idx = nxt, lvl + 1, idx // 2
                        if lvl == 5:
                            E_tiles[qt] = node
                            return
                    tree[lvl][idx] = node

                def pv(Gi):
                    qt, kc = divmod(Gi, NKC)
                    if kc == 0:
                        yaccs[qt] = ctx_ps.tile(
                            [P, NOC, QT], f32, tag="yacc", bufs=1, name="y_acc")
                    y_acc = yaccs[qt]
                    eT = exps.pop(Gi)
                    for oc in range(NOC):
                        nc.tensor.matmul(
                            y_acc[:, oc],
                            g_s[:, kc, oc * P:(oc + 1) * P],
                            eT[:],
                            start=(kc == 0),
                            stop=(kc == NKC - 1),
                            skip_group_check=True,
                        )
                    if kc == NKC - 1:
                        # release the PSUM pair promptly: cast both halves
                        yT = epool.tile([P, NOC, QT], bf16, tag="yT")
                        for oc in range(NOC):
                            nc.vector.tensor_copy(yT[:, oc], y_acc[:, oc])
                        yTs[qt] = yT

                def sums_low():
                    # first denominator half for the last q-tile, ~16 chunks
                    # before its end: shortens the serial tail
                    s_acc = ctx_ps.tile([P, QT], f32, tag="sacc", bufs=1, name="s_acc")
                    nc.tensor.matmul(
                        s_acc[:], ones_s[:], tree[4].pop(0)[:],
                        start=True, stop=False, skip_group_check=True)
                    saccs[NQT - 1] = s_acc

                def sums(qt):
                    if qt == NQT - 1:
                        s_acc = saccs[qt]
                        nc.tensor.matmul(
                            s_acc[:], ones_s[:], E_tiles.pop(qt)[:],
                            start=False, stop=True, skip_group_check=True)
                    else:
                        s_acc = ctx_ps.tile([P, QT], f32, tag="sacc", bufs=1, name="s_acc")
                        nc.tensor.matmul(
                            s_acc[:], ones_s[:], E_tiles.pop(qt)[:],
                            start=True, stop=True, skip_group_check=True)
                    saccs[qt] = s_acc  # noqa: re-store for non-last tiles
                    # 1/s = exp(-ln(s*2^-44) - 44*ln2); the 2^-44 scale is
                    # exact and recenters ln's input range (s reaches ~1e30,
                    # where the ACT ln spline misbehaves)
                    ln_t = epool.tile([P, QT], f32, tag="ln_t")
                    nc.scalar.activation(ln_t[:], s_acc[:], Ln, scale=2.0 ** -44)
                    recip = epool.tile([P, QT], f32, tag="recip")
                    nc.scalar.activation(recip[:], ln_t[:], Exp, scale=-1.0,
                                         bias=lnb_s[:, 0:1])
                    recips[qt] = recip

                def wy_out(qt, cc):
                    qsl = slice(qt * QT, (qt + 1) * QT)
                    wy = ctx_ps.tile([P, QT], f32, tag="qkv", bufs=3, name="wy")
                    yT = yTs[qt]
                    for oc in range(NOC):
                        nc.tensor.matmul(
                            wy[:],
                            wo_s[:, oc, cc * P:(cc + 1) * P],
                            yT[:, oc],
                            start=(oc == 0),
                            stop=(oc == NOC - 1),
                            skip_group_check=True,
                        )
                    ot = epool.tile([P, QT], f32, tag="ot", bufs=5)
                    nc.vector.tensor_mul(out=ot[:], in0=wy[:], in1=recips[qt][:])
                    eng = nc.gpsimd if qt == NQT - 1 else nc.vector
                    eng.tensor_add(
                        out=ot[:], in0=ot[:], in1=xq_c[cc][:, qsl])
                    nc.sync.dma_start(out_d.ap()[cc * P:(cc + 1) * P, qsl], ot[:])
                    if cc == NCC - 1:
                        yTs.pop(qt)
                        recips.pop(qt)
                        saccs.pop(qt, None)

                for G in range(TOT + LAG + 8):
                    if LAG <= G < TOT + LAG:
                        pv(G - LAG)
                    # epilogue of qtile qt, interleaved after its scores end
                    # at S = qt*NKC + (NKC-1): sums at S+3, wy at S+4..S+7
                    off = (G - (NKC - 1)) % NKC
                    qt_e = (G - (NKC - 1)) // NKC
                    if 0 <= qt_e < NQT:
                        if off == 3:
                            sums(qt_e)
                        elif 4 <= off <= 7:
                            wy_out(qt_e, off - 4)
                    if G == (NQT - 1) * NKC + NKC // 2 + 2:
                        sums_low()
                    if G < TOT:
                        scores_exp(G)
            all_ps.__exit__(None, None, None)
            xin_cm.__exit__(None, None, None)
    return nc


def _get_nc():
    if "nc" not in _cache:
        _install_bir_patch()
        _cache["nc"] = _build_nc()
    return _cache["nc"]


def kernel(x, w_theta, b_theta, w_phi, b_phi, w_g, b_g, w_out, b_out,
           _trace=False):
    import ml_dtypes
    from concourse.bass_utils import run_bass_kernel_spmd

    bf = ml_dtypes.bfloat16
    x = np.asarray(x, dtype=np.float32)
    w_theta = np.asarray(w_theta, dtype=np.float32)
    b_theta = np.asarray(b_theta, dtype=np.float32)
    w_phi = np.asarray(w_phi, dtype=np.float32)
    b_phi = np.asarray(b_phi, dtype=np.float32)
    w_g = np.asarray(w_g, dtype=np.float32)
    b_g = np.asarray(b_g, dtype=np.float32)
    w_out = np.asarray(w_out, dtype=np.float32)
    b_out = np.asarray(b_out, dtype=np.float32)

    nc = _get_nc()

    xf = np.ascontiguousarray(x.reshape(B, C, N).astype(np.float16))
    wqp = np.ascontiguousarray(
        np.concatenate([w_theta, w_phi], axis=0).T.astype(np.float16))  # [C, 2CI]
    w_c = w_phi.T @ b_theta                                     # [C]
    wg = np.ascontiguousarray(
        np.concatenate([w_g.T, w_c[:, None]], axis=1).astype(np.float16))  # [C, CI+1]
    wo = np.ascontiguousarray(w_out.T.astype(bf))               # [CI, C] bf16
    bo_eff = b_out + w_out @ b_g
    bo = np.ascontiguousarray(bo_eff.reshape(NCC, P).T)         # [P, NCC]
    ones = np.ones((P, P), dtype=bf)

    shared = {"wqp": wqp, "wg": wg, "wo": wo, "bo": bo, "ones": ones}
    in_maps = []
    for core in range(8):
        b, h = divmod(core, 2)
        # query half first; attention is permutation-invariant over keys
        xperm = np.concatenate(
            [xf[b][:, h * NQ:(h + 1) * NQ], xf[b][:, (1 - h) * NQ:(2 - h) * NQ]],
            axis=1)
        in_maps.append({"xf": np.ascontiguousarray(xperm), **shared})

    res = run_bass_kernel_spmd(nc, in_maps, core_ids=list(range(8)), trace=_trace)
    _cache["last_results"] = res

    out = np.empty((B, C, N), dtype=np.float32)
    for core in range(8):
        b, h = divmod(core, 2)
        out[b][:, h * NQ:(h + 1) * NQ] = res.results[core]["out"]
    return out.reshape(B, C, HH, WW)


# revision 14
# speedup vs baseline: 1.0180x; 1.0125x over previous
"""NonLocalBlock (dense self-attention over 64x64 pixels) on 8 Trainium2 cores.

Sharding: 8 cores = 4 batches x 2 query-halves of 2048 pixels each.
Each core holds the full x[b] (for keys/values) plus its query slice, and
computes its [C, 2048] slab of the output, residual included. The host
gathers the 8 slabs.

v2 changes over the first working kernel (235us):
  - x, QKV weights, ones ship as bf16 (halves the 8MB head DMA that kept
    PE idle for 18.7us), and ~48 tiny warmup matmuls on the ones tile run
    during the load so the HAM clock gate is at 2.4GHz when QKV starts
    (it measured 1.2GHz until 34us before).
  - theta/phi biases are never applied on device: expanding
    (th+bt)(ph+bp) gives a per-key term c[k] = (W_phi^T b_th)^T x_k
    (computed as a 257th output column of the g matmul, extracted with an
    ACT copy that also folds the -50 softmax shift), a per-QUERY term
    that softmax cancels exactly (dropped), and a constant (dropped).
  - exp output eT is bf16; the softmax denominator is a 31-add binary
    tree on DVE (bf16, 2x rate) + ONE ones-matmul per q-tile. The
    gpsimd pair/quad sums of v1 (96 adds x 1.3us = 126us busy, the
    engine was 56% occupied and stalled PE mid-tile) are gone; gpsimd
    now does nothing.
  - 1/s via reciprocal_approx_fast (51-ULP, ~5x faster than the 3.4us
    iterative reciprocal), epilogue is mul + residual-add only (bias
    pre-folded into the resident x while attention runs).
  - the per-qtile epilogue (sums matmul, out-proj, normalize, residual,
    store) is emitted interleaved into the first steps of the NEXT
    q-tile's score loop so PE never drains at tile boundaries.

Per-core math (Q=theta over its 2048 queries; phi/g over all 4096 keys):
  fT[k,q] = sum_o phi[o,k] Q[o,q]        f32r matmuls
  eT      = exp(fT + c[k] - 50)          bf16 out
  y~T[o,q]= sum_k g[k,o] eT[k,q]         bf16 matmuls, fp32 PSUM
  s[q]    = ones-matmul over tree-summed eT
  out[c,q]= (W_out^T y~T)[c,q] / s[q] + xb[c,q],  xb = x + b_out + W_out b_g
"""

import json

import numpy as np

B, C, HH, WW = 4, 512, 64, 64
CI = 256
N = HH * WW          # 4096 pixels
NQ = N // 2          # queries per core
P = 128
QT = 512             # q-tile width
NQT = NQ // QT       # 4 q-tiles per core
NKC = N // P         # 32 key chunks
NCC = C // P         # 4 channel chunks
NOC = CI // P        # 2 inter-channel chunks
SHIFT = 50.0
LAG = 3              # PV trails scores by this many chunks
NWARM = 44           # HAM warmup matmuls during the input DMA

_cache: dict = {}


def _install_bir_patch():
    """This walrus build rejects >1 sync-wait per instruction; Tile's tail
    drain (and some first-consumer instructions) carry several. Split the
    extras onto preceding single-wait EventSemaphore instructions."""
    import concourse.bass_utils as bass_utils
    import concourse.bass2jax as bass2jax

    if getattr(bass_utils.compile_bir_kernel, "_wait_split_patch", False):
        return
    orig = bass_utils.compile_bir_kernel

    def _split(bir_json: bytes) -> bytes:
        d = json.loads(bir_json)
        changed = False
        for fn in d.get("functions", []):
            for bb in fn.get("blocks", []):
                new = []
                for ins in bb.get("instructions", []):
                    si = ins.get("sync_info")
                    waits = (si or {}).get("on_wait") or []
                    if len(waits) > 1:
                        changed = True
                        for k, w in enumerate(waits[:-1]):
                            new.append({
                                "debug": ins.get("debug", 0),
                                "engine": ins["engine"],
                                "ins": [],
                                "outs": [],
                                "name": f"{ins['name']}-w{k}",
                                "opcode": "EventSemaphore",
                                "sync_info": {"on_update": [], "on_wait": [w]},
                            })
                        si["on_wait"] = [waits[-1]]
                    new.append(ins)
                bb["instructions"] = new
        return json.dumps(d).encode() if changed else bir_json

    def patched(bir_json, tmpdir, neff_name="file.neff"):
        return orig(_split(bir_json), tmpdir, neff_name)

    patched._wait_split_patch = True
    bass_utils.compile_bir_kernel = patched
    bass2jax.compile_bir_kernel = patched


def _build_nc():
    import concourse.bass as bass
    import concourse.mybir as mybir
    from concourse import tile

    dt = mybir.dt
    f32, f32r, bf16, f16 = dt.float32, dt.float32r, dt.bfloat16, dt.float16
    Exp = mybir.ActivationFunctionType.Exp
    Ln = mybir.ActivationFunctionType.Ln
    Copy = mybir.ActivationFunctionType.Copy

    nc = bass.Bass("TRN2", target_bir_lowering=False, debug=False)

    xf_d = nc.dram_tensor("xf", [C, N], f16, kind="ExternalInput")
    wqp_d = nc.dram_tensor("wqp", [C, 2 * CI], f16, kind="ExternalInput")
    wg_d = nc.dram_tensor("wg", [C, CI + 1], f16, kind="ExternalInput")
    wo_d = nc.dram_tensor("wo", [CI, C], bf16, kind="ExternalInput")
    bo_d = nc.dram_tensor("bo", [P, NCC], f32, kind="ExternalInput")
    ones_d = nc.dram_tensor("ones", [P, P], bf16, kind="ExternalInput")
    eye_d = nc.dram_tensor("eye", [P, P], f16, kind="ExternalInput")
    out_d = nc.dram_tensor("out", [C, NQ], f32, kind="ExternalOutput")

    with tile.TileContext(nc) as tc:
        with (
            tc.tile_pool(name="wts", bufs=1) as wpool,
            tc.tile_pool(name="persist", bufs=1) as ppool,
        ):
            wqp_s = wpool.tile([P, NCC, 2 * CI], f16)
            wg_s = wpool.tile([P, NCC, CI + 1], f16)
            wo_s = wpool.tile([P, NOC, C], bf16)
            bo_s = wpool.tile([P, NCC], f32)
            ones_s = wpool.tile([P, P], bf16)
            wrm_s = wpool.tile([P, P], bf16)
            eye_s = wpool.tile([P, P], f16)
            lnb_s = wpool.tile([P, 1], f32)

            q_s = ppool.tile([P, NOC, NQ], f16)
            phi_s = ppool.tile([P, NOC, N], f16)
            g_s = ppool.tile([P, NKC, CI], bf16)
            c_s = ppool.tile([P, NKC], f32)

            all_ps = tc.tile_pool(name="all_ps", bufs=1, space="PSUM")
            ctx_ps = all_ps.__enter__()

            # ---- input DMAs, warmup, QKV phase ----
            xin_cm = tc.tile_pool(name="xin", bufs=1)
            xpool = xin_cm.__enter__()
            if True:
                xf_c = [xpool.tile([P, N], f16, tag=f"xf{kc}", name=f"xf_c{kc}") for kc in range(NCC)]
                xq_c = [t[:, :NQ] for t in xf_c]
                xf_r = xf_d.ap().rearrange("(kc p) n -> kc p n", p=P)
                # ones first (warmup fodder), then the query halves
                nc.sync.dma_start(ones_s[:], ones_d.ap())
                nc.sync.dma_start(wqp_s[:], wqp_d.ap().rearrange("(kc p) m -> p kc m", p=P))
                for kc in range(NCC):
                    nc.sync.dma_start(xf_c[kc][:, :NQ], xf_r[kc][:, :NQ])
                for kc in range(NCC):
                    nc.sync.dma_start(xf_c[kc][:, NQ:], xf_r[kc][:, NQ:])
                nc.sync.dma_start(wg_s[:], wg_d.ap().rearrange("(kc p) o -> p kc o", p=P))
                nc.sync.dma_start(wo_s[:], wo_d.ap().rearrange("(oc p) c -> p oc c", p=P))
                nc.sync.dma_start(bo_s[:], bo_d.ap())
                nc.sync.dma_start(eye_s[:], eye_d.ap())

                # warm the HAM clock gate while x streams in (memset
                # fodder: no DMA dependency, PE busy from ~0)
                nc.vector.memset(wrm_s[:], 0.0)
                nc.vector.memset(lnb_s[:], -44.0 * 0.6931471805599453)
                for i in range(NWARM):
                    wp = ctx_ps.tile([P, QT], f32, tag="fps", bufs=2, name="warm")
                    nc.tensor.matmul(wp[:, :P], wrm_s[:], wrm_s[:],
                                     start=True, stop=True)

                # Q (needs only the query halves), raw: bias folded away
                for mc in range(NOC):
                    for t in range(NQ // QT):
                        ps = ctx_ps.tile([P, QT], f32, tag="qkv", bufs=3, name="ps")
                        for kc in range(NCC):
                            nc.tensor.matmul(
                                ps[:],
                                wqp_s[:, kc, mc * P:(mc + 1) * P],
                                xq_c[kc][:, t * QT:(t + 1) * QT],
                                start=(kc == 0),
                                stop=(kc == NCC - 1),
                            )
                        nc.scalar.activation(
                            q_s[:, mc, t * QT:(t + 1) * QT], ps[:], Copy)
                for mc in range(NOC):
                    for t in range(N // QT):
                        ps = ctx_ps.tile([P, QT], f32, tag="qkv", bufs=3, name="ps")
                        for kc in range(NCC):
                            nc.tensor.matmul(
                                ps[:],
                                wqp_s[:, kc, (NOC + mc) * P:(NOC + mc + 1) * P],
                                xf_c[kc][:, t * QT:(t + 1) * QT],
                                start=(kc == 0),
                                stop=(kc == NCC - 1),
                            )
                        nc.scalar.activation(
                            phi_s[:, mc, t * QT:(t + 1) * QT], ps[:], Copy)

                # g in natural [pixel, o] layout; col CI is c[k] (key-side
                # score bias), extracted with the -SHIFT fold
                for kc in range(NKC):
                    ps = ctx_ps.tile([P, QT], f32, tag="qkv", bufs=3, name="ps")[:, :CI + 1]
                    for cc in range(NCC):
                        nc.tensor.matmul(
                            ps[:],
                            xf_c[cc][:, kc * P:(kc + 1) * P],
                            wg_s[:, cc, :],
                            start=(cc == 0),
                            stop=(cc == NCC - 1),
                        )
                    nc.vector.tensor_copy(g_s[:, kc, :], ps[:, :CI])
                    nc.scalar.activation(
                        c_s[:, kc:kc + 1], ps[:, CI:CI + 1], Copy, bias=-SHIFT)

                # pre-bias the resident x query half for the residual path;
                # runs on DVE once all QKV matmul reads of xf are done
                for cc in range(NCC):
                    nc.vector.tensor_scalar_add(
                        xq_c[cc][:], xq_c[cc][:], bo_s[:, cc:cc + 1])

            # ---- attention: one global software-pipelined stream ----
            with (
                tc.tile_pool(name="attn_sb", bufs=3) as apool,
                tc.tile_pool(name="epi_sb", bufs=2) as epool,
            ):
                TOT = NQT * NKC
                exps = {}          # global chunk idx -> eT tile
                tree = [dict() for _ in range(5)]  # per qtile rebuilt
                yaccs = {}
                E_tiles = {}
                saccs = {}
                recips = {}
                yTs = {}
                wys = {}

                def scores_exp(Gi):
                    qt, kc = divmod(Gi, NKC)
                    qsl = slice(qt * QT, (qt + 1) * QT)
                    fp = ctx_ps.tile([P, QT], f32, tag="fps", bufs=2, name="fp")
                    for oc in range(NOC):
                        nc.tensor.matmul(
                            fp[:],
                            phi_s[:, oc, kc * P:(kc + 1) * P],
                            q_s[:, oc, qsl],
                            start=(oc == 0),
                            stop=(oc == NOC - 1),
                        )
                    eT = apool.tile([P, QT], bf16, tag="eT", bufs=6)
                    nc.scalar.activation(eT[:], fp[:], Exp, bias=c_s[:, kc:kc + 1])
                    exps[Gi] = eT
                    # denominator tree on DVE (bf16)
                    node, lvl = eT, 0
                    idx = kc
                    while idx % 2 == 1:
                        if lvl == 4 and qt == NQT - 1:
                            # last q-tile: the low 16-chunk half was already
                            # fed to the denominator accumulator by sums_low;
                            # keep only the high half for the closing matmul
                            E_tiles[qt] = node
                            return
                        sib = tree[lvl].pop(idx - 1) if lvl else exps[Gi - 1]
                        nxt = apool.tile([P, QT], bf16, tag=f"t{lvl}", bufs=3)
                        nc.vector.tensor_add(out=nxt[:], in0=sib[:], in1=node[:])
                        node, lvl, idx = nxt, lvl + 1, idx // 2
                        if lvl == 5:
                            E_tiles[qt] = node
                            return
                    tree[lvl][idx] = node

                def pv(Gi):
                    qt, kc = divmod(Gi, NKC)
                    if kc == 0:
                        yaccs[qt] = ctx_ps.tile(
                            [P, NOC, QT], f32, tag="yacc", bufs=1, name="y_acc")
                    y_acc = yaccs[qt]
                    eT = exps.pop(Gi)
                    for oc in range(NOC):
                        nc.tensor.matmul(
                            y_acc[:, oc],
                            g_s[:, kc, oc * P:(oc + 1) * P],
                            eT[:],
                            start=(kc == 0),
                            stop=(kc == NKC - 1),
                            skip_group_check=True,
                        )
                    if kc == NKC - 1 and qt != NQT - 1:
                        # release the PSUM pair promptly: cast both halves
                        yT = epool.tile([P, NOC, QT], bf16, tag="yT")
                        for oc in range(NOC):
                            nc.vector.tensor_copy(yT[:, oc], y_acc[:, oc])
                        yTs[qt] = yT

                def sums_low():
                    # first denominator half for the last q-tile, ~16 chunks
                    # before its end: shortens the serial tail
                    s_acc = ctx_ps.tile([P, QT], f32, tag="sacc", bufs=1, name="s_acc")
                    nc.tensor.matmul(
                        s_acc[:], ones_s[:], tree[4].pop(0)[:],
                        start=True, stop=False, skip_group_check=True)
                    saccs[NQT - 1] = s_acc

                def sums(qt):
                    if qt == NQT - 1:
                        s_acc = saccs[qt]
                        nc.tensor.matmul(
                            s_acc[:], ones_s[:], E_tiles.pop(qt)[:],
                            start=False, stop=True, skip_group_check=True)
                    else:
                        s_acc = ctx_ps.tile([P, QT], f32, tag="sacc", bufs=1, name="s_acc")
                        nc.tensor.matmul(
                            s_acc[:], ones_s[:], E_tiles.pop(qt)[:],
                            start=True, stop=True, skip_group_check=True)
                    saccs[qt] = s_acc  # noqa: re-store for non-last tiles
                    # 1/s = exp(-ln(s*2^-44) - 44*ln2); the 2^-44 scale is
                    # exact and recenters ln's input range (s reaches ~1e30,
                    # where the ACT ln spline misbehaves)
                    ln_t = epool.tile([P, QT], f32, tag="ln_t")
                    nc.scalar.activation(ln_t[:], s_acc[:], Ln, scale=2.0 ** -44)
                    recip = epool.tile([P, QT], f32, tag="recip")
                    nc.scalar.activation(recip[:], ln_t[:], Exp, scale=-1.0,
                                         bias=lnb_s[:, 0:1])
                    recips[qt] = recip

                def wy_out(qt, cc):
                    qsl = slice(qt * QT, (qt + 1) * QT)
                    last = qt == NQT - 1
                    if last and cc == 0:
                        # tail-critical: pre-normalize y so the residual can
                        # ride the out-proj PSUM group (eye @ xb) and the
                        # post-matmul work is a single cast per chunk
                        yn = epool.tile([P, NOC, QT], bf16, tag="yT")
                        for oc in range(NOC):
                            nc.vector.tensor_mul(
                                out=yn[:, oc], in0=yaccs[qt][:, oc],
                                in1=recips[qt][:])
                        yTs[qt] = yn
                    wy = ctx_ps.tile([P, QT], f32, tag="qkv", bufs=3, name="wy")
                    yT = yTs[qt]
                    if last:
                        nc.tensor.matmul(
                            wy[:], eye_s[:], xq_c[cc][:, qsl],
                            start=True, stop=False, skip_group_check=True)
                    for oc in range(NOC):
                        nc.tensor.matmul(
                            wy[:],
                            wo_s[:, oc, cc * P:(cc + 1) * P],
                            yT[:, oc],
                            start=(oc == 0 and not last),
                            stop=(oc == NOC - 1),
                            skip_group_check=True,
                        )
                    ot = epool.tile([P, QT], f32, tag="ot", bufs=5)
                    if last:
                        nc.vector.tensor_copy(ot[:], wy[:])
                    else:
                        nc.vector.tensor_mul(
                            out=ot[:], in0=wy[:], in1=recips[qt][:])
                        nc.vector.tensor_add(
                            out=ot[:], in0=ot[:], in1=xq_c[cc][:, qsl])
                    nc.sync.dma_start(out_d.ap()[cc * P:(cc + 1) * P, qsl], ot[:])
                    if cc == NCC - 1:
                        yTs.pop(qt)
                        recips.pop(qt)
                        saccs.pop(qt, None)

                for G in range(TOT + LAG + 8):
                    if LAG <= G < TOT + LAG:
                        pv(G - LAG)
                    # epilogue of qtile qt, interleaved after its scores end
                    # at S = qt*NKC + (NKC-1): sums at S+3, wy at S+4..S+7
                    off = (G - (NKC - 1)) % NKC
                    qt_e = (G - (NKC - 1)) // NKC
                    if 0 <= qt_e < NQT:
                        if off == 3:
                            sums(qt_e)
                        elif 4 <= off <= 7:
                            wy_out(qt_e, off - 4)
                    if G == (NQT - 1) * NKC + NKC // 2 + 2:
                        sums_low()
                    if G < TOT:
                        scores_exp(G)
            all_ps.__exit__(None, None, None)
            xin_cm.__exit__(None, None, None)
    return nc


def _get_nc():
    if "nc" not in _cache:
        _install_bir_patch()
        _cache["nc"] = _build_nc()
    return _cache["nc"]


def kernel(x, w_theta, b_theta, w_phi, b_phi, w_g, b_g, w_out, b_out,
           _trace=False):
    import ml_dtypes
    from concourse.bass_utils import run_bass_kernel_spmd

    bf = ml_dtypes.bfloat16
    x = np.asarray(x, dtype=np.float32)
    w_theta = np.asarray(w_theta, dtype=np.float32)
    b_theta = np.asarray(b_theta, dtype=np.float32)
    w_phi = np.asarray(w_phi, dtype=np.float32)
    b_phi = np.asarray(b_phi, dtype=np.float32)
    w_g = np.asarray(w_g, dtype=np.float32)
    b_g = np.asarray(b_g, dtype=np.float32)
    w_out = np.asarray(w_out, dtype=np.float32)
    b_out = np.asarray(b_out, dtype=np.float32)

    nc = _get_nc()

    xf = np.ascontiguousarray(x.reshape(B, C, N).astype(np.float16))
    wqp = np.ascontiguousarray(
        np.concatenate([w_theta, w_phi], axis=0).T.astype(np.float16))  # [C, 2CI]
    w_c = w_phi.T @ b_theta                                     # [C]
    wg = np.ascontiguousarray(
        np.concatenate([w_g.T, w_c[:, None]], axis=1).astype(np.float16))  # [C, CI+1]
    wo = np.ascontiguousarray(w_out.T.astype(bf))               # [CI, C] bf16
    bo_eff = b_out + w_out @ b_g
    bo = np.ascontiguousarray(bo_eff.reshape(NCC, P).T)         # [P, NCC]
    ones = np.ones((P, P), dtype=bf)
    eye = np.eye(P, dtype=np.float16)

    shared = {"wqp": wqp, "wg": wg, "wo": wo, "bo": bo, "ones": ones,
              "eye": eye}
    in_maps = []
    for core in range(8):
        b, h = divmod(core, 2)
        # query half first; attention is permutation-invariant over keys
        xperm = np.concatenate(
            [xf[b][:, h * NQ:(h + 1) * NQ], xf[b][:, (1 - h) * NQ:(2 - h) * NQ]],
            axis=1)
        in_maps.append({"xf": np.ascontiguousarray(xperm), **shared})

    res = run_bass_kernel_spmd(nc, in_maps, core_ids=list(range(8)), trace=_trace)
    _cache["last_results"] = res

    out = np.empty((B, C, N), dtype=np.float32)
    for core in range(8):
        b, h = divmod(core, 2)
        out[b][:, h * NQ:(h + 1) * NQ] = res.results[core]["out"]
    return out.reshape(B, C, HH, WW)
